# revision 3
# baseline (speedup 1.0000x reference)
"""Trainium2 Bass kernel for nn_NeRFMLPNetwork (StyleGAN-style modulated 1x1-conv MLP).

Network (per layer): s = affine(w_lat); y = conv1x1(x * s); y = y * rsqrt(demod) + b;
out = lrelu(y) * sqrt(2).  8 layers (60->128, then 7x 128->128), B=4, H*W=32768.

Strategy:
  - Data parallel over H*W: each of 8 cores handles 4096 spatial points (all batches).
  - The style path (affine styles s, demod d) is tiny (<0.01% of FLOPs) and is
    computed on the host, StyleGAN-eval style: the fully folded per-(layer,batch)
    weight  Wmod[c, o] = convT[c,o] * s[b,c] * d[b,o] * sqrt(2)  is shipped to the
    device (~2 MB), so the device does only: matmul + bias + lrelu + DMA.
  - Everything rides as bf16; PSUM stays f32.  Epilogue out = prelu(psum+gcb, 0.2)
    split between ScalarE (Prelu activation, ~995ns/1024 cols incl. PSUM access
    latency) and VectorE (custom DVE op SCALE_BIAS_LRELU, ~1252ns) -- the two
    PSUM-drain engines are the roofline (PSUM reads are 1 elem/cycle/lane/engine;
    GpSimd/DMA have no PSUM port).  Groups are assigned greedily by modeled cost
    so both engines finish together (~5:4 split).
  - 1024-col PSUM groups, 4 in flight (8 banks): drains at 2-bank granularity +
    refills at 1-bank granularity is the bank-capacity optimum; larger drain
    instructions would leave only 2 regions in flight and serialize drain/refill.
  - Head: no PE warm-up burst; first x sub-block is split into 4 column chunks
    across the sync+scalar HW-DGE queues so the first matmul starts ~1us after
    the first trigger; a dummy 1-col Prelu right after the triggers hoists the
    ACT_TABLE_LOAD (1.3us) off the first-drain critical path.
  - Tail: all DMAs ride HW-DGE queues (sync; scalar only before drains start or
    after scalar's last drain) -- the gpsimd software-DGE drain at TileContext
    exit cost ~5us in the old design.  The final sub's outputs go per-group on
    alternating queues and the very last group is drained as two 512-col halves
    by ScalarE and VectorE concurrently so both engines finish together.
"""

import numpy as np

import concourse.bacc as bacc
import concourse.mybir as mybir
import concourse.tile as tile
from concourse.bass_utils import run_bass_kernel_spmd

# ---------------------------------------------------------------------------
# Custom DVE op: out = max(z, z*imm2) with z = in0*s0 + s1   (leaky relu)
# ---------------------------------------------------------------------------
import concourse.dve_ops as dve_ops_mod
from concourse.dve_spec import Spec, Src0, C0, C1, C2, maxx, lower as _dve_lower
from concourse.dve_spec import _has_src1
from concourse.dve_uop import DveOpSpec


def _sbl_ref(in0, in1, s0, s1, imm2):
    z = in0.astype(np.float32) * s0 + s1
    return np.maximum(z, z * imm2)


_z = Src0 * C0 + C1
_SBL_SPEC = Spec(body=maxx(_z, _z * C2), reference=_sbl_ref)
SCALE_BIAS_LRELU = dve_ops_mod.DveOp(
    "SCALE_BIAS_LRELU", _SBL_SPEC, subdim=False, uops_sha={}
)
if "SCALE_BIAS_LRELU" not in dve_ops_mod._SUB_OPCODE_FOR_NAME:
    dve_ops_mod.OPS.append(SCALE_BIAS_LRELU)
    dve_ops_mod.CUSTOM_DVE_SPECS["SCALE_BIAS_LRELU"] = _SBL_SPEC
    dve_ops_mod._SUB_OPCODE_FOR_NAME["SCALE_BIAS_LRELU"] = (
        max(dve_ops_mod._SUB_OPCODE_FOR_NAME.values()) + 1
    )
for _ver in ("v3", "v4"):
    _s = DveOpSpec(
        name="SCALE_BIAS_LRELU",
        opcode=dve_ops_mod.get_dve_sub_opcode("SCALE_BIAS_LRELU"),
        uops=_dve_lower(_SBL_SPEC, ver=_ver),
        rd1_en=_has_src1(_SBL_SPEC),
    )
    SCALE_BIAS_LRELU.uops_sha[_ver] = _s.sha(_ver)

# ---------------------------------------------------------------------------
# Problem constants (hardcoded per spec)
# ---------------------------------------------------------------------------
B, CIN, H, W, HID, WDIM, NB = 4, 60, 64, 512, 128, 512, 8
HWTOT = H * W                    # 32768
N_CORES = 8
SHARD = HWTOT // N_CORES         # 4096 spatial points per core
INV_SQRT_WDIM = float(1.0 / np.sqrt(WDIM))
SQRT2 = float(np.sqrt(2.0))
EPS = 1e-8

F32 = mybir.dt.float32
BF16 = mybir.dt.bfloat16

GROUP = 1024                     # psum group columns (2 banks)
SUB = 2048                       # x columns per sub-block (striped in pairs)
NT = GROUP // 512                # matmuls per psum group
S_COST, V_COST = 995, 1252       # measured ns per 1024-col drain instr

_COMPILED = None


def _drain_assignment(n_groups):
    """Greedy S/V assignment by modeled per-instruction cost."""
    out, t_s, t_v = [], 0, 0
    for _ in range(n_groups):
        if t_s + S_COST <= t_v + V_COST:
            out.append(0)
            t_s += S_COST
        else:
            out.append(1)
            t_v += V_COST
    return out


def _build():
    nc = bacc.Bacc("TRN2", target_bir_lowering=False, debug=False,
                   num_devices=N_CORES)

    x_d = nc.dram_tensor("x", [B, CIN, SHARD], BF16, kind="ExternalInput").ap()
    wm0_d = nc.dram_tensor("wm0", [CIN, B * HID], BF16, kind="ExternalInput").ap()
    wmr_d = nc.dram_tensor("wmr", [HID, NB - 1, B * HID], BF16,
                           kind="ExternalInput").ap()
    gcb_d = nc.dram_tensor("gcb", [HID, NB], F32, kind="ExternalInput").ap()
    y_d = nc.dram_tensor("y", [B, HID, SHARD], BF16, kind="ExternalOutput").ap()

    with tile.TileContext(nc) as tc:
        with (
            tc.tile_pool(name="big", bufs=8) as big,
            tc.tile_pool(name="x0p", bufs=4) as x0p,
            tc.tile_pool(name="wts", bufs=1) as wts,
            tc.tile_pool(name="xst", bufs=4) as xst,
            tc.tile_pool(name="ps", bufs=4, space="PSUM") as ps,
        ):
            # ---- DMA order: layer-0 weight slice, first x chunks, params ----
            wm0 = wts.tile([CIN, B * HID], BF16, tag="wm0")
            nc.sync.dma_start(wm0[:, :HID], wm0_d[:, :HID])
            gcb = wts.tile([HID, NB], F32, tag="gcb")
            nc.sync.dma_start(gcb[:], gcb_d[:])

            # first sub-block (b=0, cols 0:2048) in 4 column chunks across the
            # two HW-DGE queues (scalar queue is idle until the first drain)
            x0_first = x0p.tile([CIN, SUB], BF16, tag="x0")
            for ci, eng in enumerate((nc.sync, nc.scalar, nc.sync, nc.scalar)):
                eng.dma_start(x0_first[:, ci * 512:(ci + 1) * 512],
                              x_d[0, :, ci * 512:(ci + 1) * 512])

            # hoist the Prelu ACT_TABLE_LOAD off the first-drain critical path
            ones = wts.tile([HID, 1], F32, tag="ones")
            nc.vector.memset(ones[:], 1.0)
            scratch = wts.tile([HID, 1], F32, tag="scratch")
            nc.scalar.activation(scratch[:], ones[:],
                                 mybir.ActivationFunctionType.Prelu,
                                 bias=0.0, alpha=0.2)

            nc.sync.dma_start(wm0[:, HID:], wm0_d[:, HID:])

            def load_sub(sub):
                b, col0 = sub
                if (b, col0) == (0, 0):
                    x0 = x0_first
                else:
                    x0 = x0p.tile([CIN, SUB], BF16, tag="x0")
                    nc.sync.dma_start(x0[:], x_d[b, :, col0:col0 + SUB])
                bufA = big.tile([128, SUB], BF16, tag="xbuf")
                bufB = big.tile([128, SUB], BF16, tag="xbuf")
                return x0, bufA, bufB

            subs = [(b, c0) for b in range(B) for c0 in range(0, SHARD, SUB)]
            pairs = [(subs[i], subs[i + 1]) for i in range(0, len(subs), 2)]
            bufs = {}
            for s in pairs[0]:
                bufs[s] = load_sub(s)

            # layer weights stream in behind the first x pair
            wmr = wts.tile([HID, NB - 1, B * HID], BF16, tag="wmr")
            for _l in range(NB - 1):
                nc.sync.dma_start(wmr[:, _l, :], wmr_d[:, _l, :])

            n_groups = len(pairs) * NB * 2 * (SUB // GROUP)
            assign = _drain_assignment(n_groups)

            gcnt = 0
            for pi, pair in enumerate(pairs):
                if pi + 1 < len(pairs):
                    for s in pairs[pi + 1]:
                        bufs[s] = load_sub(s)
                last_pair = pi == len(pairs) - 1
                for l in range(NB):
                    C = CIN if l == 0 else HID
                    wmod = wm0 if l == 0 else wmr[:, l - 1, :]
                    gc = gcb[:, l:l + 1]
                    last = l == NB - 1
                    for si, s in enumerate(pair):
                        b, col0 = s
                        x0, bufA, bufB = bufs[s]
                        x_in = x0 if l == 0 else (bufA if l % 2 == 1 else bufB)
                        x_out = bufA if l % 2 == 0 else bufB
                        ost = None
                        if last:
                            ost = xst.tile([128, SUB], BF16, tag="xout")
                        for g in range(SUB // GROUP):
                            pt = ps.tile([128, GROUP], F32, tag="ps")
                            c0 = g * GROUP
                            for t in range(NT):
                                nc.tensor.matmul(
                                    pt[:, t * 512:(t + 1) * 512],
                                    wmod[:C, b * HID:(b + 1) * HID],
                                    x_in[:C, c0 + t * 512:c0 + (t + 1) * 512],
                                    start=True, stop=True)
                            # epilogue: out = prelu(psum + gcb, 0.2)
                            o_full = (ost if last else x_out)[:, c0:c0 + GROUP]
                            final_group = gcnt == n_groups - 1

                            def drain(dst, src, eng):
                                if eng == 0:
                                    nc.scalar.activation(
                                        dst, src,
                                        mybir.ActivationFunctionType.Prelu,
                                        bias=gc, alpha=0.2)
                                else:
                                    nc.vector._custom_dve(
                                        SCALE_BIAS_LRELU,
                                        out=dst, in0=src,
                                        s0=ones[:, 0:1], s1=gc,
                                        imm2=0.2)

                            if final_group:
                                # split halves so both engines finish together
                                drain(o_full[:, :512], pt[:, :512], 0)
                                drain(o_full[:, 512:], pt[:, 512:], 1)
                            else:
                                drain(o_full, pt[:], assign[gcnt])
                            gcnt += 1
                            if last and last_pair:
                                # per-group outputs so data flow overlaps the
                                # remaining drains; the split final group rides
                                # both HW queues, each trigger waiting on the
                                # OTHER engine's (already final) drain
                                if final_group:
                                    half = col0 + g * GROUP
                                    nc.sync.dma_start(
                                        y_d[b, :, half:half + 512],
                                        ost[:, c0:c0 + 512])
                                    nc.scalar.dma_start(
                                        y_d[b, :, half + 512:half + GROUP],
                                        ost[:, c0 + 512:c0 + GROUP])
                                else:
                                    nc.sync.dma_start(
                                        y_d[b, :, col0 + g * GROUP:
                                            col0 + (g + 1) * GROUP],
                                        ost[:, c0:c0 + GROUP])
                        if last and not last_pair:
                            # one batched output DMA per sub on the sync queue
                            nc.sync.dma_start(
                                y_d[b, :, col0:col0 + SUB], ost[:])
                for s in pair:
                    del bufs[s]

    nc.compile()
    return nc


def _prep_inputs(pre_point_features, points_encoding, wp,
                 aff_w_in, aff_b_in, conv_w_in, conv_b_in,
                 aff_w, aff_b, conv_w, conv_b):
    """Host-side prep: layout of x + the (tiny) style path fully folded into
    per-(layer,batch) modulated-demodulated weights."""
    import ml_dtypes
    x = np.ascontiguousarray(np.asarray(points_encoding, np.float32)
                             .reshape(B, CIN, HWTOT)
                             .astype(ml_dtypes.bfloat16))
    wp = np.asarray(wp, np.float32)

    def fold(wl, aw, ab, cw):
        # wl [B,WDIM]; aw [C,WDIM]; ab [C]; cw [O,C] -> wmod [C, B*O]
        s = wl @ aw.T * INV_SQRT_WDIM + ab                      # [B, C]
        d = 1.0 / np.sqrt((s * s) @ (cw * cw).T + EPS)          # [B, O]
        wmod = (cw.T[None, :, :] * s[:, :, None] * d[:, None, :]
                * SQRT2)                                        # [B, C, O]
        return np.ascontiguousarray(
            wmod.transpose(1, 0, 2).reshape(wmod.shape[1], B * cw.shape[0]))

    aff_w_in = np.asarray(aff_w_in, np.float32)
    aff_b_in = np.asarray(aff_b_in, np.float32)
    conv_w_in = np.asarray(conv_w_in, np.float32)
    aff_w = np.asarray(aff_w, np.float32)
    aff_b = np.asarray(aff_b, np.float32)
    conv_w = np.asarray(conv_w, np.float32)

    wm0 = fold(wp[:, 0], aff_w_in, aff_b_in, conv_w_in)          # [CIN, B*HID]
    wm0 = wm0.astype(ml_dtypes.bfloat16)
    wmr = np.stack([fold(wp[:, 1 + i], aff_w[i], aff_b[i], conv_w[i])
                    for i in range(NB - 1)], axis=1)             # [HID,NB-1,B*HID]
    wmr = np.ascontiguousarray(wmr.astype(ml_dtypes.bfloat16))

    gcb = np.empty((HID, NB), np.float32)
    gcb[:, 0] = SQRT2 * np.asarray(conv_b_in, np.float32)
    gcb[:, 1:] = SQRT2 * np.asarray(conv_b, np.float32).T

    shared = dict(wm0=wm0, wmr=wmr, gcb=gcb)
    in_maps = []
    for c in range(N_CORES):
        m = dict(shared)
        m["x"] = np.ascontiguousarray(x[:, :, c * SHARD:(c + 1) * SHARD])
        in_maps.append(m)
    return in_maps


def kernel(trace=False, **inputs):
    global _COMPILED
    if _COMPILED is None:
        _COMPILED = _build()
    nc = _COMPILED
    in_maps = _prep_inputs(**inputs)
    res = run_bass_kernel_spmd(nc, in_maps, core_ids=list(range(N_CORES)),
                               trace=trace)
    parts = [np.asarray(res.results[c]["y"]).astype(np.float32)
             for c in range(N_CORES)]
    out = np.concatenate(parts, axis=2).reshape(B, HID, H, W)
    if trace:
        kernel.last_result = res
    return out


# revision 5
# speedup vs baseline: 1.0086x; 1.0086x over previous
"""Trainium2 Bass kernel for nn_NeRFMLPNetwork (StyleGAN-style modulated 1x1-conv MLP).

Network (per layer): s = affine(w_lat); y = conv1x1(x * s); y = y * rsqrt(demod) + b;
out = lrelu(y) * sqrt(2).  8 layers (60->128, then 7x 128->128), B=4, H*W=32768.

Strategy:
  - Data parallel over H*W: each of 8 cores handles 4096 spatial points (all batches).
  - The style path (affine styles s, demod d) is tiny (<0.01% of FLOPs) and is
    computed on the host, StyleGAN-eval style: the fully folded per-(layer,batch)
    weight  Wmod[c, o] = convT[c,o] * s[b,c] * d[b,o] * sqrt(2)  is shipped to the
    device (~2 MB), so the device does only: matmul + bias + lrelu + DMA.
  - Everything rides as bf16; PSUM stays f32.  Epilogue out = prelu(psum+gcb, 0.2)
    split between ScalarE (Prelu activation, ~995ns/1024 cols incl. PSUM access
    latency) and VectorE (custom DVE op SCALE_BIAS_LRELU, ~1252ns) -- the two
    PSUM-drain engines are the roofline (PSUM reads are 1 elem/cycle/lane/engine;
    GpSimd/DMA have no PSUM port).  Groups are assigned greedily by modeled cost
    so both engines finish together (~5:4 split).
  - 1024-col PSUM groups, 4 in flight (8 banks): drains at 2-bank granularity +
    refills at 1-bank granularity is the bank-capacity optimum; larger drain
    instructions would leave only 2 regions in flight and serialize drain/refill.
  - Head: no PE warm-up burst; first x sub-block is split into 4 column chunks
    across the sync+scalar HW-DGE queues so the first matmul starts ~1us after
    the first trigger; a dummy 1-col Prelu right after the triggers hoists the
    ACT_TABLE_LOAD (1.3us) off the first-drain critical path.
  - Tail: all DMAs ride HW-DGE queues (sync; scalar only before drains start or
    after scalar's last drain) -- the gpsimd software-DGE drain at TileContext
    exit cost ~5us in the old design.  The final sub's outputs go per-group on
    alternating queues and the very last group is drained as two 512-col halves
    by ScalarE and VectorE concurrently so both engines finish together.
"""

import numpy as np

import concourse.bacc as bacc
import concourse.mybir as mybir
import concourse.tile as tile
from concourse.bass_utils import run_bass_kernel_spmd

# ---------------------------------------------------------------------------
# Custom DVE op: out = max(z, z*imm2) with z = in0*s0 + s1   (leaky relu)
# ---------------------------------------------------------------------------
import concourse.dve_ops as dve_ops_mod
from concourse.dve_spec import Spec, Src0, C0, C1, C2, maxx, lower as _dve_lower
from concourse.dve_spec import _has_src1
from concourse.dve_uop import DveOpSpec


def _sbl_ref(in0, in1, s0, s1, imm2):
    z = in0.astype(np.float32) * s0 + s1
    return np.maximum(z, z * imm2)


_z = Src0 * C0 + C1
_SBL_SPEC = Spec(body=maxx(_z, _z * C2), reference=_sbl_ref)
SCALE_BIAS_LRELU = dve_ops_mod.DveOp(
    "SCALE_BIAS_LRELU", _SBL_SPEC, subdim=False, uops_sha={}
)
if "SCALE_BIAS_LRELU" not in dve_ops_mod._SUB_OPCODE_FOR_NAME:
    dve_ops_mod.OPS.append(SCALE_BIAS_LRELU)
    dve_ops_mod.CUSTOM_DVE_SPECS["SCALE_BIAS_LRELU"] = _SBL_SPEC
    dve_ops_mod._SUB_OPCODE_FOR_NAME["SCALE_BIAS_LRELU"] = (
        max(dve_ops_mod._SUB_OPCODE_FOR_NAME.values()) + 1
    )
for _ver in ("v3", "v4"):
    _s = DveOpSpec(
        name="SCALE_BIAS_LRELU",
        opcode=dve_ops_mod.get_dve_sub_opcode("SCALE_BIAS_LRELU"),
        uops=_dve_lower(_SBL_SPEC, ver=_ver),
        rd1_en=_has_src1(_SBL_SPEC),
    )
    SCALE_BIAS_LRELU.uops_sha[_ver] = _s.sha(_ver)

# ---------------------------------------------------------------------------
# Problem constants (hardcoded per spec)
# ---------------------------------------------------------------------------
B, CIN, H, W, HID, WDIM, NB = 4, 60, 64, 512, 128, 512, 8
HWTOT = H * W                    # 32768
N_CORES = 8
SHARD = HWTOT // N_CORES         # 4096 spatial points per core
INV_SQRT_WDIM = float(1.0 / np.sqrt(WDIM))
SQRT2 = float(np.sqrt(2.0))
EPS = 1e-8

F32 = mybir.dt.float32
F32R = mybir.dt.float32r
BF16 = mybir.dt.bfloat16

GROUP = 1024                     # psum group columns (2 banks)
SUB = 2048                     # x columns per sub-block (striped in pairs)
NT = GROUP // 512                # matmuls per psum group
S_COST, V_COST = 1000, 1260      # measured ns per 1024-col drain instr
WARMUP = 7                       # dummy MMs at t=0 to un-throttle the PE clock

_COMPILED = None


def _drain_assignment(n_groups):
    """Greedy S/V assignment by modeled per-instruction cost."""
    out, t_s, t_v = [], 0, 0
    for _ in range(n_groups):
        if t_s + S_COST <= t_v + V_COST:
            out.append(0)
            t_s += S_COST
        else:
            out.append(1)
            t_v += V_COST
    return out


def _build():
    nc = bacc.Bacc("TRN2", target_bir_lowering=False, debug=False,
                   num_devices=N_CORES)

    x_d = nc.dram_tensor("x", [B, CIN, SHARD], BF16, kind="ExternalInput").ap()
    wm0_d = nc.dram_tensor("wm0", [CIN, B * HID], BF16, kind="ExternalInput").ap()
    wmr_d = nc.dram_tensor("wmr", [HID, NB - 1, B * HID], BF16,
                           kind="ExternalInput").ap()
    gcb_d = nc.dram_tensor("gcb", [HID, NB], F32, kind="ExternalInput").ap()
    y_d = nc.dram_tensor("y", [B, HID, SHARD], BF16, kind="ExternalOutput").ap()

    with tile.TileContext(nc) as tc:
        with (
            tc.tile_pool(name="big", bufs=8) as big,
            tc.tile_pool(name="x0p", bufs=4) as x0p,
            tc.tile_pool(name="wts", bufs=1) as wts,
            tc.tile_pool(name="xst", bufs=4) as xst,
            tc.tile_pool(name="ps", bufs=4, space="PSUM") as ps,
        ):
            # ---- head: first x chunk on sync, small params on gpsimd, the
            # Prelu ACT_TABLE_LOAD hoisted onto the (otherwise idle) scalar
            # queue, and a short PE warm-up burst so HAM reaches the 2.4 GHz
            # pstate before the first real matmul (~3.2us of PE activity).
            x0_first = x0p.tile([CIN, SUB], BF16, tag="x0")
            nc.sync.dma_start(x0_first[:, :GROUP], x_d[0, :, :GROUP])
            wm0 = wts.tile([CIN, B * HID], BF16, tag="wm0")
            nc.gpsimd.dma_start(wm0[:, :HID], wm0_d[:, :HID])
            gcb = wts.tile([HID, NB], F32, tag="gcb")
            nc.gpsimd.dma_start(gcb[:], gcb_d[:])
            nc.gpsimd.dma_start(x0_first[:, GROUP:], x_d[0, :, GROUP:SUB])

            ones = wts.tile([HID, 1], F32, tag="ones")
            nc.vector.memset(ones[:], 1.0)
            scratch = wts.tile([HID, 1], F32, tag="scratch")
            nc.scalar.activation(scratch[:], ones[:],
                                 mybir.ActivationFunctionType.Prelu,
                                 bias=0.0, alpha=0.2)

            wrm = wts.tile([128, 512], F32, tag="wrm")
            nc.vector.memset(wrm[:], 0.5)
            for _i in range(WARMUP):
                ptw = ps.tile([128, GROUP], F32, tag="ps")
                nc.tensor.matmul(ptw[:, :512], wrm[:, :128].bitcast(F32R),
                                 wrm[:].bitcast(F32R), start=True, stop=True)

            def load_sub(sub, eng=None):
                b, col0 = sub
                if (b, col0) == (0, 0):
                    x0 = x0_first
                else:
                    x0 = x0p.tile([CIN, SUB], BF16, tag="x0")
                    (eng or nc.sync).dma_start(x0[:], x_d[b, :, col0:col0 + SUB])
                bufA = big.tile([128, SUB], BF16, tag="xbuf")
                bufB = big.tile([128, SUB], BF16, tag="xbuf")
                return x0, bufA, bufB

            subs = [(b, c0) for b in range(B) for c0 in range(0, SHARD, SUB)]
            pairs = [(subs[i], subs[i + 1]) for i in range(0, len(subs), 2)]
            bufs = {}
            bufs[pairs[0][0]] = load_sub(pairs[0][0])
            bufs[pairs[0][1]] = load_sub(pairs[0][1], nc.gpsimd)

            nc.sync.dma_start(wm0[:, HID:], wm0_d[:, HID:])
            # layer weights stream in behind the first x pair
            wmr = wts.tile([HID, NB - 1, B * HID], BF16, tag="wmr")
            for _l in range(NB - 1):
                nc.sync.dma_start(wmr[:, _l, :], wmr_d[:, _l, :])

            n_groups = len(pairs) * NB * 2 * (SUB // GROUP)
            assign = _drain_assignment(n_groups)

            gcnt = 0
            for pi, pair in enumerate(pairs):
                if pi + 1 < len(pairs):
                    for s in pairs[pi + 1]:
                        bufs[s] = load_sub(s)
                last_pair = pi == len(pairs) - 1
                for l in range(NB):
                    C = CIN if l == 0 else HID
                    wmod = wm0 if l == 0 else wmr[:, l - 1, :]
                    gc = gcb[:, l:l + 1]
                    last = l == NB - 1
                    for si, s in enumerate(pair):
                        b, col0 = s
                        x0, bufA, bufB = bufs[s]
                        x_in = x0 if l == 0 else (bufA if l % 2 == 1 else bufB)
                        x_out = bufA if l % 2 == 0 else bufB
                        ost = None
                        if last:
                            ost = xst.tile([128, SUB], BF16, tag="xout")
                        for g in range(SUB // GROUP):
                            pt = ps.tile([128, GROUP], F32, tag="ps")
                            c0 = g * GROUP
                            for t in range(NT):
                                nc.tensor.matmul(
                                    pt[:, t * 512:(t + 1) * 512],
                                    wmod[:C, b * HID:(b + 1) * HID],
                                    x_in[:C, c0 + t * 512:c0 + (t + 1) * 512],
                                    start=True, stop=True)
                            # epilogue: out = prelu(psum + gcb, 0.2)
                            o_full = (ost if last else x_out)[:, c0:c0 + GROUP]
                            final_group = gcnt == n_groups - 1

                            def drain(dst, src, eng):
                                if eng == 0:
                                    nc.scalar.activation(
                                        dst, src,
                                        mybir.ActivationFunctionType.Prelu,
                                        bias=gc, alpha=0.2)
                                else:
                                    nc.vector._custom_dve(
                                        SCALE_BIAS_LRELU,
                                        out=dst, in0=src,
                                        s0=ones[:, 0:1], s1=gc,
                                        imm2=0.2)

                            if final_group:
                                # split halves so both engines finish together
                                drain(o_full[:, :512], pt[:, :512], 0)
                                drain(o_full[:, 512:], pt[:, 512:], 1)
                            else:
                                drain(o_full, pt[:], assign[gcnt])
                            gcnt += 1
                            if last and last_pair:
                                # per-group outputs so data flow overlaps the
                                # remaining drains; the split final group rides
                                # both HW queues, each trigger waiting on the
                                # OTHER engine's (already final) drain
                                if final_group:
                                    half = col0 + g * GROUP
                                    nc.sync.dma_start(
                                        y_d[b, :, half:half + 512],
                                        ost[:, c0:c0 + 512])
                                    nc.scalar.dma_start(
                                        y_d[b, :, half + 512:half + GROUP],
                                        ost[:, c0 + 512:c0 + GROUP])
                                else:
                                    nc.sync.dma_start(
                                        y_d[b, :, col0 + g * GROUP:
                                            col0 + (g + 1) * GROUP],
                                        ost[:, c0:c0 + GROUP])
                        if last and not last_pair:
                            # one batched output DMA per sub on the sync queue
                            nc.sync.dma_start(
                                y_d[b, :, col0:col0 + SUB], ost[:])
                for s in pair:
                    del bufs[s]

    nc.compile()
    return nc


def _prep_inputs(pre_point_features, points_encoding, wp,
                 aff_w_in, aff_b_in, conv_w_in, conv_b_in,
                 aff_w, aff_b, conv_w, conv_b):
    """Host-side prep: layout of x + the (tiny) style path fully folded into
    per-(layer,batch) modulated-demodulated weights."""
    import ml_dtypes
    x = np.ascontiguousarray(np.asarray(points_encoding, np.float32)
                             .reshape(B, CIN, HWTOT)
                             .astype(ml_dtypes.bfloat16))
    wp = np.asarray(wp, np.float32)

    def fold(wl, aw, ab, cw):
        # wl [B,WDIM]; aw [C,WDIM]; ab [C]; cw [O,C] -> wmod [C, B*O]
        s = wl @ aw.T * INV_SQRT_WDIM + ab                      # [B, C]
        d = 1.0 / np.sqrt((s * s) @ (cw * cw).T + EPS)          # [B, O]
        wmod = (cw.T[None, :, :] * s[:, :, None] * d[:, None, :]
                * SQRT2)                                        # [B, C, O]
        return np.ascontiguousarray(
            wmod.transpose(1, 0, 2).reshape(wmod.shape[1], B * cw.shape[0]))

    aff_w_in = np.asarray(aff_w_in, np.float32)
    aff_b_in = np.asarray(aff_b_in, np.float32)
    conv_w_in = np.asarray(conv_w_in, np.float32)
    aff_w = np.asarray(aff_w, np.float32)
    aff_b = np.asarray(aff_b, np.float32)
    conv_w = np.asarray(conv_w, np.float32)

    wm0 = fold(wp[:, 0], aff_w_in, aff_b_in, conv_w_in)          # [CIN, B*HID]
    wm0 = wm0.astype(ml_dtypes.bfloat16)
    wmr = np.stack([fold(wp[:, 1 + i], aff_w[i], aff_b[i], conv_w[i])
                    for i in range(NB - 1)], axis=1)             # [HID,NB-1,B*HID]
    wmr = np.ascontiguousarray(wmr.astype(ml_dtypes.bfloat16))

    gcb = np.empty((HID, NB), np.float32)
    gcb[:, 0] = SQRT2 * np.asarray(conv_b_in, np.float32)
    gcb[:, 1:] = SQRT2 * np.asarray(conv_b, np.float32).T

    shared = dict(wm0=wm0, wmr=wmr, gcb=gcb)
    in_maps = []
    for c in range(N_CORES):
        m = dict(shared)
        m["x"] = np.ascontiguousarray(x[:, :, c * SHARD:(c + 1) * SHARD])
        in_maps.append(m)
    return in_maps


def kernel(trace=False, **inputs):
    global _COMPILED
    if _COMPILED is None:
        _COMPILED = _build()
    nc = _COMPILED
    in_maps = _prep_inputs(**inputs)
    res = run_bass_kernel_spmd(nc, in_maps, core_ids=list(range(N_CORES)),
                               trace=trace)
    parts = [np.asarray(res.results[c]["y"]).astype(np.float32)
             for c in range(N_CORES)]
    out = np.concatenate(parts, axis=2).reshape(B, HID, H, W)
    if trace:
        kernel.last_result = res
    return out


# revision 8
# speedup vs baseline: 1.0181x; 1.0094x over previous
"""Trainium2 Bass kernel for nn_NeRFMLPNetwork (StyleGAN-style modulated 1x1-conv MLP).

Network (per layer): s = affine(w_lat); y = conv1x1(x * s); y = y * rsqrt(demod) + b;
out = lrelu(y) * sqrt(2).  8 layers (60->128, then 7x 128->128), B=4, H*W=32768.

Strategy:
  - Data parallel over H*W: each of 8 cores handles 4096 spatial points (all batches).
  - The style path (affine styles s, demod d) is tiny (<0.01% of FLOPs) and is
    computed on the host, StyleGAN-eval style: the fully folded per-(layer,batch)
    weight  Wmod[c, o] = convT[c,o] * s[b,c] * d[b,o] * sqrt(2)  is shipped to the
    device (~2 MB), so the device does only: matmul + bias + lrelu + DMA.
  - Everything rides as bf16; PSUM stays f32.  Epilogue out = prelu(psum+gcb, 0.2)
    split between ScalarE (Prelu activation, ~995ns/1024 cols incl. PSUM access
    latency) and VectorE (custom DVE op SCALE_BIAS_LRELU, ~1252ns) -- the two
    PSUM-drain engines are the roofline (PSUM reads are 1 elem/cycle/lane/engine;
    GpSimd/DMA have no PSUM port).  Groups are assigned greedily by modeled cost
    so both engines finish together (~5:4 split).
  - 1024-col PSUM groups, 4 in flight (8 banks): drains at 2-bank granularity +
    refills at 1-bank granularity is the bank-capacity optimum; larger drain
    instructions would leave only 2 regions in flight and serialize drain/refill.
  - Head: no PE warm-up burst; first x sub-block is split into 4 column chunks
    across the sync+scalar HW-DGE queues so the first matmul starts ~1us after
    the first trigger; a dummy 1-col Prelu right after the triggers hoists the
    ACT_TABLE_LOAD (1.3us) off the first-drain critical path.
  - Tail: all DMAs ride HW-DGE queues (sync; scalar only before drains start or
    after scalar's last drain) -- the gpsimd software-DGE drain at TileContext
    exit cost ~5us in the old design.  The final sub's outputs go per-group on
    alternating queues and the very last group is drained as two 512-col halves
    by ScalarE and VectorE concurrently so both engines finish together.
"""

import numpy as np

import concourse.bacc as bacc
import concourse.mybir as mybir
import concourse.tile as tile
from concourse.bass_utils import run_bass_kernel_spmd

# ---------------------------------------------------------------------------
# Custom DVE op: out = max(z, z*imm2) with z = in0*s0 + s1   (leaky relu)
# ---------------------------------------------------------------------------
import concourse.dve_ops as dve_ops_mod
from concourse.dve_spec import Spec, Src0, C0, C1, C2, maxx, lower as _dve_lower
from concourse.dve_spec import _has_src1
from concourse.dve_uop import DveOpSpec


def _sbl_ref(in0, in1, s0, s1, imm2):
    z = in0.astype(np.float32) * s0 + s1
    return np.maximum(z, z * imm2)


_z = Src0 * C0 + C1
_SBL_SPEC = Spec(body=maxx(_z, _z * C2), reference=_sbl_ref)
SCALE_BIAS_LRELU = dve_ops_mod.DveOp(
    "SCALE_BIAS_LRELU", _SBL_SPEC, subdim=False, uops_sha={}
)
if "SCALE_BIAS_LRELU" not in dve_ops_mod._SUB_OPCODE_FOR_NAME:
    dve_ops_mod.OPS.append(SCALE_BIAS_LRELU)
    dve_ops_mod.CUSTOM_DVE_SPECS["SCALE_BIAS_LRELU"] = _SBL_SPEC
    dve_ops_mod._SUB_OPCODE_FOR_NAME["SCALE_BIAS_LRELU"] = (
        max(dve_ops_mod._SUB_OPCODE_FOR_NAME.values()) + 1
    )
for _ver in ("v3", "v4"):
    _s = DveOpSpec(
        name="SCALE_BIAS_LRELU",
        opcode=dve_ops_mod.get_dve_sub_opcode("SCALE_BIAS_LRELU"),
        uops=_dve_lower(_SBL_SPEC, ver=_ver),
        rd1_en=_has_src1(_SBL_SPEC),
    )
    SCALE_BIAS_LRELU.uops_sha[_ver] = _s.sha(_ver)

# ---------------------------------------------------------------------------
# Problem constants (hardcoded per spec)
# ---------------------------------------------------------------------------
B, CIN, H, W, HID, WDIM, NB = 4, 60, 64, 512, 128, 512, 8
HWTOT = H * W                    # 32768
N_CORES = 8
SHARD = HWTOT // N_CORES         # 4096 spatial points per core
INV_SQRT_WDIM = float(1.0 / np.sqrt(WDIM))
SQRT2 = float(np.sqrt(2.0))
EPS = 1e-8

F32 = mybir.dt.float32
F32R = mybir.dt.float32r
BF16 = mybir.dt.bfloat16

GROUP = 1024                     # psum group columns (2 banks)
SUB = 2048                     # x columns per sub-block (striped in pairs)
NT = GROUP // 512                # matmuls per psum group
S_COST, V_COST = 1000, 1265      # measured ns per 1024-col drain instr
WARMUP = 6                       # dummy MMs at t=0 to un-throttle the PE clock

_COMPILED = None


def _drain_assignment(n_groups):
    """Greedy S/V assignment by modeled per-instruction cost."""
    out, t_s, t_v = [], 0, 0
    for _ in range(n_groups):
        if t_s + S_COST <= t_v + V_COST:
            out.append(0)
            t_s += S_COST
        else:
            out.append(1)
            t_v += V_COST
    return out


def _build():
    nc = bacc.Bacc("TRN2", target_bir_lowering=False, debug=False,
                   num_devices=N_CORES)

    x_d = nc.dram_tensor("x", [B, CIN, SHARD], BF16, kind="ExternalInput").ap()
    wm0_d = nc.dram_tensor("wm0", [CIN, B * HID], BF16, kind="ExternalInput").ap()
    wmr_d = nc.dram_tensor("wmr", [HID, NB - 1, B * HID], BF16,
                           kind="ExternalInput").ap()
    gcb_d = nc.dram_tensor("gcb", [HID, NB], F32, kind="ExternalInput").ap()
    y_d = nc.dram_tensor("y", [B, HID, SHARD], BF16, kind="ExternalOutput").ap()

    with tile.TileContext(nc) as tc:
        with (
            tc.tile_pool(name="big", bufs=8) as big,
            tc.tile_pool(name="x0p", bufs=4) as x0p,
            tc.tile_pool(name="wts", bufs=1) as wts,
            tc.tile_pool(name="xst", bufs=4) as xst,
            tc.tile_pool(name="ps", bufs=4, space="PSUM") as ps,
        ):
            # ---- head.  HAM starts the core half-throttled, lifts the limit
            # after ~3.4us of continuous PE activity, and RE-throttles after
            # any sizeable PE idle gap -- so the warm-up burst must bridge
            # seamlessly into the real matmul stream, which in turn needs the
            # first x chunks resident before the burst ends.  Trigger
            # instructions cost ~700ns on their issuing queue, so the first
            # loads are spread over sync (x chunks), gpsimd (small params,
            # software-DGE) and scalar (idle until the first drain).
            x0_first = x0p.tile([CIN, SUB], BF16, tag="x0")
            x0_second = x0p.tile([CIN, SUB], BF16, tag="x0")
            for h in range(2):      # first sub-block, 1024-col chunks on sync
                nc.sync.dma_start(x0_first[:, h * GROUP:(h + 1) * GROUP],
                                  x_d[0, :, h * GROUP:(h + 1) * GROUP])
            wm0 = wts.tile([CIN, B * HID], BF16, tag="wm0")
            nc.gpsimd.dma_start(wm0[:, :HID], wm0_d[:, :HID])
            gcb = wts.tile([HID, NB], F32, tag="gcb")
            nc.gpsimd.dma_start(gcb[:], gcb_d[:])

            # hoist the Prelu ACT_TABLE_LOAD off the first-drain critical path
            ones = wts.tile([HID, 1], F32, tag="ones")
            nc.vector.memset(ones[:], 1.0)
            scratch = wts.tile([HID, 1], F32, tag="scratch")
            nc.scalar.activation(scratch[:], ones[:],
                                 mybir.ActivationFunctionType.Prelu,
                                 bias=0.0, alpha=0.2)

            wrm = wts.tile([128, 512], F32, tag="wrm")
            nc.vector.memset(wrm[:], 0.5)
            for _i in range(WARMUP):
                ptw = ps.tile([128, GROUP], F32, tag="ps")
                nc.tensor.matmul(ptw[:, :512], wrm[:, :128].bitcast(F32R),
                                 wrm[:].bitcast(F32R), start=True, stop=True)

            for h in range(2):      # second sub-block chunks behind the first
                nc.sync.dma_start(x0_second[:, h * GROUP:(h + 1) * GROUP],
                                  x_d[0, :, SUB + h * GROUP:SUB + (h + 1) * GROUP])

            wmr = wts.tile([HID, NB - 1, B * HID], BF16, tag="wmr")
            # first two layer weights ride the still-idle scalar queue
            nc.scalar.dma_start(wmr[:, 0, :], wmr_d[:, 0, :])
            nc.scalar.dma_start(wmr[:, 1, :], wmr_d[:, 1, :])
            nc.sync.dma_start(wm0[:, HID:], wm0_d[:, HID:])
            nc.sync.dma_start(wmr[:, 2, :], wmr_d[:, 2, :])
            # wmr layers 3-6 are issued inside the pair-0 body, after the
            # pair-1 x prefetch triggers

            def load_sub(sub):
                b, col0 = sub
                if (b, col0) == (0, 0):
                    x0 = x0_first
                elif (b, col0) == (0, SUB):
                    x0 = x0_second
                else:
                    x0 = x0p.tile([CIN, SUB], BF16, tag="x0")
                    nc.sync.dma_start(x0[:], x_d[b, :, col0:col0 + SUB])
                bufA = big.tile([128, SUB], BF16, tag="xbuf")
                bufB = big.tile([128, SUB], BF16, tag="xbuf")
                return x0, bufA, bufB

            subs = [(b, c0) for b in range(B) for c0 in range(0, SHARD, SUB)]
            pairs = [(subs[i], subs[i + 1]) for i in range(0, len(subs), 2)]
            bufs = {}
            for s in pairs[0]:
                bufs[s] = load_sub(s)

            n_groups = len(pairs) * NB * 2 * (SUB // GROUP)
            assign = _drain_assignment(n_groups)

            gcnt = 0
            for pi, pair in enumerate(pairs):
                if pi + 1 < len(pairs):
                    for s in pairs[pi + 1]:
                        bufs[s] = load_sub(s)
                if pi == 0:
                    for _l in range(3, NB - 1):
                        nc.sync.dma_start(wmr[:, _l, :], wmr_d[:, _l, :])
                last_pair = pi == len(pairs) - 1
                for l in range(NB):
                    C = CIN if l == 0 else HID
                    wmod = wm0 if l == 0 else wmr[:, l - 1, :]
                    gc = gcb[:, l:l + 1]
                    last = l == NB - 1
                    for si, s in enumerate(pair):
                        b, col0 = s
                        x0, bufA, bufB = bufs[s]
                        x_in = x0 if l == 0 else (bufA if l % 2 == 1 else bufB)
                        x_out = bufA if l % 2 == 0 else bufB
                        ost = None
                        if last:
                            ost = xst.tile([128, SUB], BF16, tag="xout")
                        for g in range(SUB // GROUP):
                            pt = ps.tile([128, GROUP], F32, tag="ps")
                            c0 = g * GROUP
                            for t in range(NT):
                                nc.tensor.matmul(
                                    pt[:, t * 512:(t + 1) * 512],
                                    wmod[:C, b * HID:(b + 1) * HID],
                                    x_in[:C, c0 + t * 512:c0 + (t + 1) * 512],
                                    start=True, stop=True)
                            # epilogue: out = prelu(psum + gcb, 0.2)
                            o_full = (ost if last else x_out)[:, c0:c0 + GROUP]
                            final_group = gcnt == n_groups - 1

                            def drain(dst, src, eng):
                                if eng == 0:
                                    nc.scalar.activation(
                                        dst, src,
                                        mybir.ActivationFunctionType.Prelu,
                                        bias=gc, alpha=0.2)
                                else:
                                    nc.vector._custom_dve(
                                        SCALE_BIAS_LRELU,
                                        out=dst, in0=src,
                                        s0=ones[:, 0:1], s1=gc,
                                        imm2=0.2)

                            if final_group:
                                # split halves so both engines finish together
                                drain(o_full[:, :512], pt[:, :512], 0)
                                drain(o_full[:, 512:], pt[:, 512:], 1)
                            else:
                                drain(o_full, pt[:], assign[gcnt])
                            gcnt += 1
                            if last and last_pair:
                                # per-group outputs so data flow overlaps the
                                # remaining drains; the split final group rides
                                # both HW queues, each trigger waiting on the
                                # OTHER engine's (already final) drain
                                if final_group:
                                    half = col0 + g * GROUP
                                    nc.sync.dma_start(
                                        y_d[b, :, half:half + 512],
                                        ost[:, c0:c0 + 512])
                                    nc.scalar.dma_start(
                                        y_d[b, :, half + 512:half + GROUP],
                                        ost[:, c0 + 512:c0 + GROUP])
                                else:
                                    nc.sync.dma_start(
                                        y_d[b, :, col0 + g * GROUP:
                                            col0 + (g + 1) * GROUP],
                                        ost[:, c0:c0 + GROUP])
                        if last and not last_pair:
                            # one batched output DMA per sub on the sync queue
                            nc.sync.dma_start(
                                y_d[b, :, col0:col0 + SUB], ost[:])
                for s in pair:
                    del bufs[s]

    nc.compile()
    return nc


def _prep_inputs(pre_point_features, points_encoding, wp,
                 aff_w_in, aff_b_in, conv_w_in, conv_b_in,
                 aff_w, aff_b, conv_w, conv_b):
    """Host-side prep: layout of x + the (tiny) style path fully folded into
    per-(layer,batch) modulated-demodulated weights."""
    import ml_dtypes
    x = np.ascontiguousarray(np.asarray(points_encoding, np.float32)
                             .reshape(B, CIN, HWTOT)
                             .astype(ml_dtypes.bfloat16))
    wp = np.asarray(wp, np.float32)

    def fold(wl, aw, ab, cw):
        # wl [B,WDIM]; aw [C,WDIM]; ab [C]; cw [O,C] -> wmod [C, B*O]
        s = wl @ aw.T * INV_SQRT_WDIM + ab                      # [B, C]
        d = 1.0 / np.sqrt((s * s) @ (cw * cw).T + EPS)          # [B, O]
        wmod = (cw.T[None, :, :] * s[:, :, None] * d[:, None, :]
                * SQRT2)                                        # [B, C, O]
        return np.ascontiguousarray(
            wmod.transpose(1, 0, 2).reshape(wmod.shape[1], B * cw.shape[0]))

    aff_w_in = np.asarray(aff_w_in, np.float32)
    aff_b_in = np.asarray(aff_b_in, np.float32)
    conv_w_in = np.asarray(conv_w_in, np.float32)
    aff_w = np.asarray(aff_w, np.float32)
    aff_b = np.asarray(aff_b, np.float32)
    conv_w = np.asarray(conv_w, np.float32)

    wm0 = fold(wp[:, 0], aff_w_in, aff_b_in, conv_w_in)          # [CIN, B*HID]
    wm0 = wm0.astype(ml_dtypes.bfloat16)
    wmr = np.stack([fold(wp[:, 1 + i], aff_w[i], aff_b[i], conv_w[i])
                    for i in range(NB - 1)], axis=1)             # [HID,NB-1,B*HID]
    wmr = np.ascontiguousarray(wmr.astype(ml_dtypes.bfloat16))

    gcb = np.empty((HID, NB), np.float32)
    gcb[:, 0] = SQRT2 * np.asarray(conv_b_in, np.float32)
    gcb[:, 1:] = SQRT2 * np.asarray(conv_b, np.float32).T

    shared = dict(wm0=wm0, wmr=wmr, gcb=gcb)
    in_maps = []
    for c in range(N_CORES):
        m = dict(shared)
        m["x"] = np.ascontiguousarray(x[:, :, c * SHARD:(c + 1) * SHARD])
        in_maps.append(m)
    return in_maps


def kernel(trace=False, **inputs):
    global _COMPILED
    if _COMPILED is None:
        _COMPILED = _build()
    nc = _COMPILED
    in_maps = _prep_inputs(**inputs)
    res = run_bass_kernel_spmd(nc, in_maps, core_ids=list(range(N_CORES)),
                               trace=trace)
    parts = [np.asarray(res.results[c]["y"]).astype(np.float32)
             for c in range(N_CORES)]
    out = np.concatenate(parts, axis=2).reshape(B, HID, H, W)
    if trace:
        kernel.last_result = res
    return out
